# revision 2
# baseline (speedup 1.0000x reference)
"""Trainium2 Bass kernel for nn_ButterflyLayer.

Reference computation:
    h   = x @ w_in.T                       [B, 2048]
    h   = butterfly(h, a_pad, b_pad)       11 stages of paired rotations
    out = h @ w_out.T + b_out              [B, 2048]

Key algebraic facts used here:
  * The butterfly is a linear map B on the 2048-dim:  out = x @ (w_out @ B @ w_in).T + b.
  * B factors as (M (x) I_128) @ blockdiag(D_0..D_15) where
      - D_c (128x128) is the composition of stages 0..6 restricted to 128-chunk c
        (those stages never mix across 128-aligned chunks), and
      - stages 7..10 use one scalar coefficient per 128-chunk, so they act as a
        16x16 matrix M on chunk indices, identically for every position inside
        a chunk.
  * So W_eff = (w_out @ (M (x) I)) @ blockdiag(D) @ w_in, and the butterfly
    costs only a block-sparse (128-wide) contraction instead of a dense one.

Host prep is limited to O(dim^2) parameter/layout work: expanding the tiny
rotation params (a_pad/b_pad) into the D_c blocks, folding the 16x16 chunk mix
M into w_out, and transposing operands into the PE-friendly layouts. All
O(batch*dim^2) compute runs on the NeuronCores.

Device program (SPMD on 8 cores, 4 batch-groups x 2 out-column-groups):
  build:  g1[c]   = D_c^T-transform of the core's w_out' column slice   (16+16 mm)
          W_effT  = w_in-contraction of g1                              (512 mm)
  main:   outT    = W_effT^T @ xT (+ bias), streamed over batch         (1024 mm)
All matmuls use float32r (TF32-like fast fp32, 1 cycle/row at N=512).
"""

import sys

if "/opt/trn_rl_repo" not in sys.path:
    sys.path.insert(0, "/opt/trn_rl_repo")

import numpy as np

import concourse.bass as bass
import concourse.mybir as mybir
import concourse.tile as tile
from concourse import bacc
from concourse.bass import ts
from concourse.bass_utils import run_bass_kernel_spmd

DIM = 2048
LOG_DIM = 11
BATCH = 16384
N_CORES = 8
GB = 4                 # batch groups
GN = 2                 # output-column groups
MSH = BATCH // GB      # 4096 batch rows per core
NSL = DIM // GN        # 1024 output columns per core
P = 128                # partitions
NB = 512               # matmul moving free dim (one PSUM bank of fp32)
NCHUNK = DIM // P      # 16
F32 = mybir.dt.float32
F32R = mybir.dt.float32r


# ---------------------------------------------------------------- host math

def _butterfly_dense(a_pad, b_pad, stages):
    """Dense matrix of the butterfly restricted to `stages` (float64).

    Returns Bm with butterfly(v) = Bm @ v for v in R^DIM.
    """
    x = np.eye(DIM, dtype=np.float64)  # rows: basis vectors
    for l in stages:
        bs = 1 << l
        nb = DIM // (2 * bs)
        a = a_pad[l, :nb].astype(np.float64)[None, :, None]
        b = b_pad[l, :nb].astype(np.float64)[None, :, None]
        xv = x.reshape(DIM, nb, 2, bs)
        x0 = xv[:, :, 0, :]
        x1 = xv[:, :, 1, :]
        top = a * x0 + b * x1
        bot = -b * x0 + a * x1
        x = np.stack([top, bot], axis=2).reshape(DIM, DIM)
    return x.T  # butterfly(I)[r] = Bm @ e_r, so butterfly(I) = Bm.T


def _host_prep(x, w_in, w_out, b_out, a_pad, b_pad):
    """Expand butterfly params and lay out operands for the device program."""
    d_full = _butterfly_dense(a_pad, b_pad, range(7))          # blockdiag(D_c)
    m_full = _butterfly_dense(a_pad, b_pad, range(7, LOG_DIM))  # M (x) I_128
    m_small = np.ascontiguousarray(m_full[::P, ::P])            # [16, 16]

    # dblk[c*128+k, j] = D_c[k, j]
    dblk = np.stack(
        [d_full[c * P:(c + 1) * P, c * P:(c + 1) * P] for c in range(NCHUNK)]
    ).reshape(DIM, P).astype(np.float32)

    # w_out' = w_out @ (M (x) I):  w_out'[:, c'*128+j] = sum_c M[c,c'] w_out[:, c*128+j]
    w_out64 = w_out.astype(np.float64).reshape(DIM, NCHUNK, P)
    w_out_p = np.einsum("icj,cd->idj", w_out64, m_small).reshape(DIM, DIM)
    wopT = np.ascontiguousarray(w_out_p.T).astype(np.float32)   # [k, n]

    xT = np.ascontiguousarray(x.T)                              # [d, batch]
    w_in_c = np.ascontiguousarray(w_in)                         # [f, d]
    bias = np.ascontiguousarray(b_out.reshape(DIM, 1)).astype(np.float32)
    return xT, w_in_c, wopT, dblk, bias


# ------------------------------------------------------------- device build

def _build_nc():
    nc = bacc.Bacc("TRN2", target_bir_lowering=False, debug=False,
                   num_devices=N_CORES)

    xt = nc.dram_tensor("xt", [DIM, MSH], F32R, kind="ExternalInput")
    w_in = nc.dram_tensor("w_in", [DIM, DIM], F32R, kind="ExternalInput")
    wop = nc.dram_tensor("wop", [DIM, NSL], F32R, kind="ExternalInput")
    dblk = nc.dram_tensor("dblk", [DIM, P], F32R, kind="ExternalInput")
    bias = nc.dram_tensor("bias", [NSL, 1], F32, kind="ExternalInput")
    outt = nc.dram_tensor("outt", [NSL, MSH], F32, kind="ExternalOutput")

    n_nb = NSL // NB        # 512-wide column blocks of the n-slice (2)
    n_mb = MSH // NB        # 512-wide batch blocks (8)
    n_nt = NSL // P         # 128-wide n tiles (8)

    with tile.TileContext(nc) as tc:
        with (
            tc.tile_pool(name="geom", bufs=1) as geom,          # persistent
            tc.tile_pool(name="psum", bufs=8, space="PSUM") as psum,
        ):
            # --- persistent tiles
            dblk_sb = geom.tile([P, DIM], F32R)      # 16 D_c blocks side by side
            for c in range(NCHUNK):
                nc.sync.dma_start(out=dblk_sb[:, ts(c, P)],
                                  in_=dblk[c * P:(c + 1) * P, :])
            bias_sb = geom.tile([P, n_nt], F32)
            for nt in range(n_nt):
                nc.sync.dma_start(out=bias_sb[:, nt:nt + 1],
                                  in_=bias[nt * P:(nt + 1) * P, :])
            weff_sb = [geom.tile([P, NSL], F32R, name=f"weff{dt}")
                       for dt in range(NCHUNK)]

            # --- build W_effT = (w_in.T @ blockdiag(D).T @ w_out'.T)[:, n-slice]
            with tc.tile_pool(name="bld", bufs=3) as bld, \
                 tc.tile_pool(name="g1p", bufs=1) as g1p:
                g1_sb = [g1p.tile([P, NSL], F32R, name=f"g1_{c}")
                         for c in range(NCHUNK)]
                # g1[c] = D_c^T-transform of w_out'^T chunk c
                for nb in range(n_nb):
                    for c in range(NCHUNK):
                        g0 = bld.tile([P, NB], F32R, tag="g0")
                        nc.sync.dma_start(
                            out=g0,
                            in_=wop[c * P:(c + 1) * P, ts(nb, NB)])
                        pt = psum.tile([P, NB], F32, tag="ps")
                        nc.tensor.matmul(pt[:, :], dblk_sb[:, ts(c, P)], g0,
                                         start=True, stop=True)
                        nc.any.tensor_copy(g1_sb[c][:, ts(nb, NB)], pt[:, :])
                # W_effT[dt] = sum_ft w_in[ft, dt].T @ g1[ft]
                for dt in range(NCHUNK):
                    wslab = bld.tile([P, DIM], F32R, tag="wslab")
                    for ft in range(NCHUNK):
                        nc.sync.dma_start(
                            out=wslab[:, ts(ft, P)],
                            in_=w_in[ft * P:(ft + 1) * P, dt * P:(dt + 1) * P])
                    for nb in range(n_nb):
                        pt = psum.tile([P, NB], F32, tag="ps")
                        for ft in range(NCHUNK):
                            nc.tensor.matmul(pt[:, :], wslab[:, ts(ft, P)],
                                             g1_sb[ft][:, ts(nb, NB)],
                                             start=(ft == 0),
                                             stop=(ft == NCHUNK - 1))
                        nc.any.tensor_copy(weff_sb[dt][:, ts(nb, NB)], pt[:, :])

            # --- main: outT[nt, mb] = sum_dt W_effT[dt, nt].T @ xT[dt, mb] + bias
            with tc.tile_pool(name="mn", bufs=2) as mn, \
                 tc.tile_pool(name="ob", bufs=4) as ob:
                for mb in range(n_mb):
                    xs = mn.tile([P, NCHUNK * NB], F32R, tag="xs")
                    for dt in range(NCHUNK):
                        nc.sync.dma_start(
                            out=xs[:, ts(dt, NB)],
                            in_=xt[dt * P:(dt + 1) * P, ts(mb, NB)])
                    for nt in range(n_nt):
                        pt = psum.tile([P, NB], F32, tag="ps")
                        for dt in range(NCHUNK):
                            nc.tensor.matmul(pt[:, :],
                                             weff_sb[dt][:, ts(nt, P)],
                                             xs[:, ts(dt, NB)],
                                             start=(dt == 0),
                                             stop=(dt == NCHUNK - 1))
                        osb = ob.tile([P, NB], F32, tag="osb")
                        nc.scalar.activation(
                            osb, pt[:, :],
                            mybir.ActivationFunctionType.Identity,
                            bias=bias_sb[:, nt:nt + 1])
                        nc.sync.dma_start(
                            out=outt[nt * P:(nt + 1) * P, ts(mb, NB)],
                            in_=osb)

    nc.compile()
    return nc


_NC_CACHE = None


def _get_nc():
    global _NC_CACHE
    if _NC_CACHE is None:
        _NC_CACHE = _build_nc()
    return _NC_CACHE


# ------------------------------------------------------------------ driver

def _make_in_maps(x, w_in, w_out, b_out, a_pad, b_pad):
    xT, w_in_c, wopT, dblk, bias = _host_prep(x, w_in, w_out, b_out,
                                              a_pad, b_pad)
    in_maps = []
    for core in range(N_CORES):
        b, g = divmod(core, GN)
        in_maps.append({
            "xt": np.ascontiguousarray(xT[:, b * MSH:(b + 1) * MSH]),
            "w_in": w_in_c,
            "wop": np.ascontiguousarray(wopT[:, g * NSL:(g + 1) * NSL]),
            "dblk": dblk,
            "bias": np.ascontiguousarray(bias[g * NSL:(g + 1) * NSL]),
        })
    return in_maps


def _assemble(results):
    out = np.empty((BATCH, DIM), dtype=np.float32)
    for core in range(N_CORES):
        b, g = divmod(core, GN)
        out[b * MSH:(b + 1) * MSH, g * NSL:(g + 1) * NSL] = \
            results[core]["outt"].T
    return out


def kernel(x, w_in, w_out, b_out, a_pad, b_pad, _trace=False):
    nc = _get_nc()
    in_maps = _make_in_maps(x, w_in, w_out, b_out, a_pad, b_pad)
    res = run_bass_kernel_spmd(nc, in_maps, core_ids=list(range(N_CORES)),
                               trace=_trace)
    out = _assemble(res.results)
    if _trace:
        kernel.last_result = res
    return out
